# revision 1
# baseline (speedup 1.0000x reference)
"""Trainium2 Bass kernel for AliceAttention (dense transformer attention layer).

Sharding: tensor-parallel over the 32 heads -> 4 heads per core across 8
NeuronCores; each core emits a partial o_proj (y_c = ao_c @ Wo[:, cols_c].T)
in fp16 and the 8 partials are summed on the host in fp32.

Layout/engine choices (v2):
  * fp16 everywhere (same PE rate as bf16, ~6x lower rounding error).
  * RoPE rotate_half is done on DVE with partition-swapped reads and a
    sign-folded sin table (no PE permutation matmul).
  * Scores are computed transposed, scores_T = [t_k, t_q]; exp runs on ACT
    with bias -12 (softmax is shift-invariant; keeps fp16 sums in range).
  * Softmax denominators: exp tiles are accumulated on DVE in fp16
    (two-accumulator chains), one ones-matmul per (pair, q-block) reduces
    over partitions; 1/sum via reciprocal_approx_fast.
  * Phase overlap: b=1 projections interleave with b=0 attention;
    o_proj tiles interleave with b=1 attention; all spill/load DMAs use
    blocked DRAM layouts with >=1KB contiguous lines.
"""

import numpy as np
import ml_dtypes
from contextlib import ExitStack

BF = ml_dtypes.bfloat16

import orjson

import concourse.bass as bass
import concourse.mybir as mybir
import concourse.tile as tile
import concourse.bass2jax as bass2jax
from concourse.bass_utils import run_bass_kernel_spmd

# ─────────────────────────────────────────────────────────────────────────
# Walrus in this container rejects instructions carrying more semaphore
# waits than their ISA struct can hold. Split excess waits into wait-only
# EventSemaphore instructions on the same engine — semantically identical.
# ─────────────────────────────────────────────────────────────────────────
_WAIT_CAP = {"EventSemaphore": 2}
_DEFAULT_WAIT_CAP = 1


def _legalize_bir_waits(bir_bytes: bytes) -> bytes:
    d = orjson.loads(bir_bytes)
    changed = False
    for fn in d.get("functions", []):
        for blk in fn.get("blocks", []):
            insts = blk.get("instructions")
            if not insts:
                continue
            out = []
            for inst in insts:
                si = inst.get("sync_info")
                waits = (si or {}).get("on_wait") or []
                cap = _WAIT_CAP.get(inst.get("opcode"), _DEFAULT_WAIT_CAP)
                if len(waits) > cap:
                    excess, keep = waits[:-cap], waits[-cap:]
                    for i in range(0, len(excess), 2):
                        out.append(
                            {
                                "debug": inst.get("debug"),
                                "engine": inst["engine"],
                                "ins": [],
                                "outs": [],
                                "name": f"{inst['name']}_xw{i}",
                                "opcode": "EventSemaphore",
                                "sync_info": {
                                    "on_update": [],
                                    "on_wait": excess[i : i + 2],
                                },
                            }
                        )
                    si["on_wait"] = keep
                    changed = True
                out.append(inst)
            blk["instructions"] = out
    return orjson.dumps(d) if changed else bir_bytes


if not getattr(bass2jax, "_wait_legalize_patched", False):
    _orig_compile_bir_kernel = bass2jax.compile_bir_kernel

    def _patched_compile_bir_kernel(ant_bir_str, compile_dir_path, **kw):
        return _orig_compile_bir_kernel(
            _legalize_bir_waits(ant_bir_str), compile_dir_path, **kw
        )

    bass2jax.compile_bir_kernel = _patched_compile_bir_kernel
    bass2jax._wait_legalize_patched = True

# ─────────────────────────────────────────────────────────────────────────
# Problem constants (hardcoded per contract)
# ─────────────────────────────────────────────────────────────────────────
B, S, H, NH, HD = 2, 2048, 4096, 32, 128
THETA = 10000.0
NCORES = 8
HPC = NH // NCORES          # heads per core = 4
OC = HPC * HD               # output cols per core = 512
T = B * S                   # 4096 tokens
KT = H // 128               # 32 contraction tiles for projections
KTQ = KT // 4               # 8 tiles per x quarter-block
TB = 512                    # t-block width in phase A
NTB = T // TB               # 8 t-blocks (0-3 = b0, 4-7 = b1)
NQ = S // 512               # 4 query blocks per pair
NK = S // 128               # 16 key tiles per pair
SCALE = 1.0 / float(np.sqrt(HD))
EXP_BIAS = -12.0            # softmax shift; cancels in the normalization

F32 = mybir.dt.float32
F16 = mybir.dt.bfloat16  # device 16-bit dtype: bf16 (fp16 runs 0.8x on PE)
EXPF = mybir.ActivationFunctionType.Exp
LNF = mybir.ActivationFunctionType.Ln


def _kept(j, mode):
    if mode == "causal":
        return list(range(min(NK, 4 * j + 4)))
    return list(range(NK))


def _build(mode: str) -> bass.Bass:
    """mode: 'causal' (skip masked tiles, 4 diag patterns),
    'zeros' (no mask, all tiles), 'general' (stream fp16 mask tiles)."""
    nc = bass.Bass()

    # blocked layouts (host-prepared):
    #   xtb [128, tb*KT*TB]   x[(k*128+p), tb*512+t] at col ((tb*KT+k)*TB+t)
    #   wq/wk/wv [128, KT*OC] W.T[(k*128+p), oc] at col (k*OC+oc)
    #   wo [128, HPC*H]       Wo[:,osl].T[(hl*128+p), n] at col (hl*H+n)
    xtb = nc.declare_dram_parameter("xtb", [128, NTB * KT * TB], F16, isOutput=False)
    wq = nc.declare_dram_parameter("wq", [128, KT * OC], F16, isOutput=False)
    wk = nc.declare_dram_parameter("wk", [128, KT * OC], F16, isOutput=False)
    wv = nc.declare_dram_parameter("wv", [128, KT * OC], F16, isOutput=False)
    wo = nc.declare_dram_parameter("wo", [128, HPC * H], F16, isOutput=False)
    cost = nc.declare_dram_parameter("cost", [HD, S], F16, isOutput=False)
    sinp = nc.declare_dram_parameter("sinp", [HD, S], F16, isOutput=False)
    ones_t = nc.declare_dram_parameter("ones_t", [128, 128], F16, isOutput=False)
    pt = nc.declare_dram_parameter("pt", [HD, HD], F16, isOutput=False)
    if mode == "causal":
        mdiag = nc.declare_dram_parameter("mdiag", [128, 512], F16, isOutput=False)
    elif mode == "general":
        maskt = nc.declare_dram_parameter("maskt", [S, S], F16, isOutput=False)
    y = nc.declare_dram_parameter("y", [T, H], F16, isOutput=True)

    # DRAM scratch: roped qT/kT per head [128, T] rows hl*128+p, col = t;
    # v blocked the same way: row hl*128+p, col = global k-tile*128 + d
    qts = nc.dram_tensor("qts", [OC, T], F16)
    kts = nc.dram_tensor("kts", [OC, T], F16)
    vts = nc.dram_tensor("vts", [OC, T], F16)
    aots = nc.dram_tensor("aots", [OC, T], F16)

    # register the exp bias constant (activation() needs a const AP for it)
    _bias_t = nc.alloc_sbuf_tensor("const-exp-bias", [128, 1], F32)
    nc.gpsimd.memset(_bias_t.ap(), EXP_BIAS)
    nc.const_aps.aps[(F32, EXP_BIAS)] = _bias_t.ap()

    with tile.TileContext(nc) as tc, ExitStack() as octx:
        const_pool = octx.enter_context(tc.tile_pool(name="const", bufs=1))
        qk_pool = octx.enter_context(tc.tile_pool(name="qkv_pair", bufs=2))
        exp_pool = octx.enter_context(tc.tile_pool(name="exp", bufs=3))
        acc_pool = octx.enter_context(tc.tile_pool(name="acc", bufs=2))
        nrm_pool = octx.enter_context(tc.tile_pool(name="nrm", bufs=1))
        asp_pool = octx.enter_context(tc.tile_pool(name="aosp", bufs=2))
        st_pool = octx.enter_context(tc.tile_pool(name="stage", bufs=3))
        if mode == "general":
            mt_pool = octx.enter_context(tc.tile_pool(name="mtile", bufs=4))
        sc_pool = octx.enter_context(tc.tile_pool(name="psSc", bufs=2, space="PSUM"))
        av_pool = octx.enter_context(tc.tile_pool(name="psAv", bufs=2, space="PSUM"))
        sm_pool = octx.enter_context(tc.tile_pool(name="psSum", bufs=1, space="PSUM"))

        blk_unlocked = []  # (b, j) blocks whose aots spills are emitted

        # ═══════ attention chain (pair-major), as a generator ═══════
        def load_pair(b, hl):
            osl = slice(hl * 128, (hl + 1) * 128)
            bsl = slice(b * S, (b + 1) * S)
            qT = qk_pool.tile([HD, S], F16, tag="qT", name=f"qT{b}{hl}")
            kT = qk_pool.tile([HD, S], F16, tag="kT", name=f"kT{b}{hl}")
            v = qk_pool.tile([128, NK * HD], F16, tag="v", name=f"v{b}{hl}")
            nc.sync.dma_start(out=kT[:], in_=kts[osl, bsl])
            nc.sync.dma_start(out=qT[:], in_=qts[osl, bsl])
            nc.sync.dma_start(out=v[:], in_=vts[osl, bsl])
            return qT, kT, v

        def attn_chain(order):
            """Pair-major attention over the given (b, hl) pairs. First yield
            emits only the first two pair loads (prime it early; all spills
            for these batches must already be emitted - DRAM RAW deps are
            emission-ordered); later yields are one ki-step each."""
            pending = [load_pair(*order[0]), load_pair(*order[1])]
            yield  # prime point: loads emitted, no compute yet
            for pi, (b, hl) in enumerate(order):
                qT, kT, v = pending.pop(0)
                if pi + 2 < len(order):
                    pending.append(load_pair(*order[pi + 2]))
                steps = [(j, ki) for j in range(NQ) for ki in _kept(j, mode)]
                prev = None  # (j, ki, exp_sb, first, last)
                avps = {}
                accs = {}

                def finish(stp, b=b, hl=hl, v=v, avps=avps, accs=accs):
                    j, ki, exp_sb, first, last = stp
                    nc.tensor.matmul(
                        avps[j][:],
                        v[:, ki * HD : (ki + 1) * HD],
                        exp_sb[:],
                        start=first,
                        stop=last,
                    )
                    if last:
                        a0, a1, cnt = accs[j]
                        if cnt > 1:
                            nc.vector.tensor_add(a0[:], a0[:], a1[:])
                        sm_ps = sm_pool.tile([128, 512], F32, tag="sm")
                        nc.tensor.matmul(
                            sm_ps[:], ones_sb[:], a0[:], start=True, stop=True
                        )
                        ln_sb = nrm_pool.tile([128, 512], F32, tag="lnv")
                        nc.scalar.activation(ln_sb[:], sm_ps[:], LNF)
                        rc = nrm_pool.tile([128, 512], F32, tag="rc")
                        nc.scalar.activation(rc[:], ln_sb[:], EXPF, scale=-1.0)
                        sp = asp_pool.tile([128, 512], F16, tag="aosp", name="sp")
                        nc.vector.tensor_mul(sp[:], avps[j][:], rc[:])
                        nc.sync.dma_start(
                            out=aots[
                                hl * 128 : (hl + 1) * 128,
                                b * S + j * 512 : b * S + (j + 1) * 512,
                            ],
                            in_=sp[:],
                        )
                        if hl == HPC - 1:
                            blk_unlocked.append((b, j))

                for j, ki in steps:
                    kept = _kept(j, mode)
                    first, last = ki == kept[0], ki == kept[-1]
                    if first:
                        avps[j] = av_pool.tile(
                            [128, 512], F32, tag="av", name=f"av{pi}_{j}"
                        )
                        accs[j] = [None, None, 0]
                    sc_ps = sc_pool.tile([128, 512], F32, tag="sc")
                    nc.tensor.matmul(
                        sc_ps[:],
                        kT[:, ki * 128 : (ki + 1) * 128],
                        qT[:, j * 512 : (j + 1) * 512],
                        start=True,
                        stop=True,
                    )
                    exp_sb = exp_pool.tile([128, 512], F16, tag="exp")
                    nc.scalar.activation(
                        exp_sb[:], sc_ps[:], EXPF, scale=SCALE, bias=EXP_BIAS
                    )
                    if mode == "causal" and ki >= 4 * j:
                        r = ki - 4 * j
                        w = 512 - r * 128
                        if r > 0:
                            nc.vector.memset(exp_sb[:, : r * 128], 0.0)
                        nc.vector.tensor_mul(
                            exp_sb[:, r * 128 :],
                            exp_sb[:, r * 128 :],
                            md_sb[:, :w],
                        )
                    elif mode == "general":
                        m_sb = mt_pool.tile([128, 512], F16, tag="mt")
                        nc.sync.dma_start(
                            out=m_sb[:],
                            in_=maskt[
                                ki * 128 : (ki + 1) * 128, j * 512 : (j + 1) * 512
                            ],
                        )
                        nc.vector.tensor_mul(exp_sb[:], exp_sb[:], m_sb[:])
                    # bf16 two-accumulator chain for the softmax denominator
                    a = accs[j]
                    w = a[2] % 2
                    if a[2] < 2:
                        t_acc = acc_pool.tile(
                            [128, 512], F16, tag=f"acc{w}", name=f"acc{pi}_{j}_{w}"
                        )
                        nc.vector.tensor_copy(t_acc[:], exp_sb[:])
                        a[w] = t_acc
                    else:
                        nc.vector.tensor_add(a[w][:], a[w][:], exp_sb[:])
                    a[2] += 1
                    # delayed AV for the previous step (hides exp latency)
                    if prev is not None:
                        finish(prev)
                    prev = (j, ki, exp_sb, first, last)
                    yield
                finish(prev)
                yield

        # ═══════ o_proj tile (stationaries staged from aots per j-block) ═══════
        oproj_count = [0]

        def stage_block(b, jb):
            st = {}
            for hl in range(HPC):
                t_st = st_pool.tile(
                    [128, 512], F16, tag=f"st{hl}", name=f"st{b}{jb}{hl}"
                )
                nc.sync.dma_start(
                    out=t_st[:],
                    in_=aots[
                        hl * 128 : (hl + 1) * 128,
                        b * S + jb * 512 : b * S + (jb + 1) * 512,
                    ],
                )
                st[hl] = t_st
            return st

        def emit_oproj(st, b, mt, n):
            msl = slice(b * S + mt * 128, b * S + (mt + 1) * 128)
            ps = yp_pool.tile([128, 512], F32, tag="y")
            for hl in range(HPC):
                nc.tensor.matmul(
                    ps[:],
                    st[hl][:, (mt % 4) * 128 : (mt % 4 + 1) * 128],
                    wo_sb[:, hl * H + n * 512 : hl * H + (n + 1) * 512],
                    start=(hl == 0),
                    stop=(hl == HPC - 1),
                )
            y_sb = yo_pool.tile([128, 512], F16, tag="ysb")
            if oproj_count[0] % 2 == 0:
                nc.scalar.copy(y_sb[:], ps[:])
            else:
                nc.vector.tensor_copy(y_sb[:], ps[:])
            oproj_count[0] += 1
            nc.sync.dma_start(out=y[msl, n * 512 : (n + 1) * 512], in_=y_sb[:])

        # ═══════ Phase A (+R2): projections + RoPE, spill to DRAM ═══════
        with ExitStack() as actx:
            w_pool = actx.enter_context(tc.tile_pool(name="wqk", bufs=1))
            x_pool = actx.enter_context(tc.tile_pool(name="xblk", bufs=2))
            ev_pool = actx.enter_context(tc.tile_pool(name="evac", bufs=2))
            rp_pool = actx.enter_context(tc.tile_pool(name="rope", bufs=2))
            vh_pool = actx.enter_context(tc.tile_pool(name="vhl", bufs=1))
            ps_pool = actx.enter_context(
                tc.tile_pool(name="psA", bufs=2, space="PSUM")
            )
            rot_pool = actx.enter_context(
                tc.tile_pool(name="psRot", bufs=1, space="PSUM")
            )

            wq_sb = w_pool.tile([128, KT * OC], F16, tag="wq")
            wk_sb = w_pool.tile([128, KT * OC], F16, tag="wk")
            wv_sb = w_pool.tile([128, KT * OC], F16, tag="wv")

            def load_x_quarter(tb, qi):
                xh = x_pool.tile(
                    [128, KTQ * TB],
                    F16,
                    tag=f"xq{qi}",
                    name=f"x{tb}{qi}",
                    bufs=2 if qi == 0 else 1,
                )
                base = (tb * KT + qi * KTQ) * TB
                nc.sync.dma_start(out=xh[:], in_=xtb[:, base : base + KTQ * TB])
                return xh

            # start-ramp ordering: wq halves + x(tb0) quarters first
            HW = KT * OC // 2
            nc.sync.dma_start(out=wq_sb[:, :HW], in_=wq[:, :HW])
            x0q = [load_x_quarter(0, 0)]
            cos_sb = const_pool.tile([HD, S], F16)
            sin_sb = const_pool.tile([HD, S], F16)
            nc.sync.dma_start(out=cos_sb[:], in_=cost[:])
            nc.sync.dma_start(out=sin_sb[:], in_=sinp[:])
            x0q.append(load_x_quarter(0, 1))
            nc.sync.dma_start(out=wq_sb[:, HW:], in_=wq[:, HW:])
            x0q += [load_x_quarter(0, 2), load_x_quarter(0, 3)]
            nc.sync.dma_start(out=wk_sb[:, :HW], in_=wk[:, :HW])

            ones_sb = const_pool.tile([128, 128], F16)
            nc.sync.dma_start(out=ones_sb[:], in_=ones_t[:])
            pt_sb = const_pool.tile([HD, HD], F16)
            nc.sync.dma_start(out=pt_sb[:], in_=pt[:])
            if mode == "causal":
                md_sb = const_pool.tile([128, 512], F16)
                nc.sync.dma_start(out=md_sb[:], in_=mdiag[:])

            nc.sync.dma_start(out=wk_sb[:, HW:], in_=wk[:, HW:])
            nc.sync.dma_start(out=wv_sb[:, :HW], in_=wv[:, :HW])
            nc.sync.dma_start(out=wv_sb[:, HW:], in_=wv[:, HW:])

            def emit_tb(tb, xq=None):
                """Projections for t-block tb; yields after each of 13 groups."""
                if xq is None:
                    xq = [load_x_quarter(tb, qi) for qi in range(4)]
                xh = xq
                tsl = slice((tb % 4) * TB, (tb % 4 + 1) * TB)  # cos/sin cols
                gsl = slice(tb * TB, (tb + 1) * TB)            # global t cols
                def _qk_groups():
                    for which, w_sb, spill in (("q", wq_sb, qts), ("k", wk_sb, kts)):
                        for ot in range(HPC):
                            ps = ps_pool.tile([128, TB], F32, tag="proj")
                            for k in range(KT):
                                nc.tensor.matmul(
                                    ps[:],
                                    w_sb[:, k * OC + ot * 128 : k * OC + (ot + 1) * 128],
                                    xh[k // KTQ][:, (k % KTQ) * TB : (k % KTQ + 1) * TB],
                                    start=(k == 0),
                                    stop=(k == KT - 1),
                                )
                            raw = ev_pool.tile([128, TB], F16, tag="raw")
                            nc.scalar.copy(raw[:], ps[:])
                            # RoPE: rotate_half via PE permutation matmul
                            rot_ps = rot_pool.tile([128, TB], F32, tag="rot")
                            nc.tensor.matmul(
                                rot_ps[:], pt_sb[:], raw[:], start=True, stop=True
                            )
                            t1 = rp_pool.tile([128, TB], F16, tag="t1")
                            nc.vector.tensor_mul(t1[:], raw[:], cos_sb[:, tsl])
                            t2 = rp_pool.tile([128, TB], F16, tag="t2")
                            nc.vector.tensor_mul(t2[:], rot_ps[:], sin_sb[:, tsl])
                            nc.vector.tensor_add(raw[:], t1[:], t2[:])
                            nc.sync.dma_start(
                                out=spill[ot * 128 : (ot + 1) * 128, gsl], in_=raw[:]
                            )
                            yield
                def _v_groups():
                    vhl = [
                        vh_pool.tile([128, TB], F16, tag=f"vhl{hl}", name=f"vhl{hl}")
                        for hl in range(HPC)
                    ]
                    for mt in range(TB // 128):
                        ps = ps_pool.tile([128, OC], F32, tag="proj", name="psv")
                        for k in range(KT):
                            nc.tensor.matmul(
                                ps[:],
                                xh[k // KTQ][
                                    :, (k % KTQ) * TB + mt * 128 : (k % KTQ) * TB + (mt + 1) * 128
                                ],
                                wv_sb[:, k * OC : (k + 1) * OC],
                                start=(k == 0),
                                stop=(k == KT - 1),
                            )
                        for hl in range(HPC):
                            nc.scalar.copy(
                                vhl[hl][:, mt * 128 : (mt + 1) * 128],
                                ps[:, hl * 128 : (hl + 1) * 128],
                            )
                        yield
                    for hl in range(HPC):
                        nc.sync.dma_start(
                            out=vts[hl * 128 : (hl + 1) * 128, gsl], in_=vhl[hl][:]
                        )
                segs = (_v_groups, _qk_groups) if tb == 7 else (_qk_groups, _v_groups)
                for _seg in segs:
                    yield from _seg()

            PAIR_CHUNKS = sum(len(_kept(j, mode)) for j in range(NQ)) + 1
            B0_CHUNKS = HPC * PAIR_CHUNKS
            ag = attn_chain([(0, hl) for hl in range(HPC)])
            chunks = 0

            def pump(n_target):
                nonlocal chunks
                while chunks < n_target:
                    try:
                        next(ag)
                    except StopIteration:
                        return False
                    chunks += 1
                return True

            # A1: b=0 projections, dense; prime pair loads at the tail
            for tb in range(4):
                g = emit_tb(tb, x0q if tb == 0 else None)
                for _ in g:
                    pass
            next(ag)  # prime: emits first two pair loads only

            # R2: b=1 projections interleaved with b=0 attention
            groups = 0
            for tb in range(4, 8):
                for _ in emit_tb(tb):
                    groups += 1
                    pump(min((B0_CHUNKS * groups) // (4 * 12) + 1, B0_CHUNKS))
            # all b=1 spills are emitted now: safe to prime the b=1 chain;
            # its pair-0/1 loads overlap the b=0 attention drain below
            bg = attn_chain([(1, hl) for hl in range(HPC)])
            next(bg)

        # ═══════ R3: o_proj interleaved with remaining attention ═══════
        wo_pool = octx.enter_context(tc.tile_pool(name="wo", bufs=1))
        yo_pool = octx.enter_context(tc.tile_pool(name="yout", bufs=3))
        yp_pool = octx.enter_context(tc.tile_pool(name="psY", bufs=3, space="PSUM"))

        wo_sb = wo_pool.tile([128, HPC * H], F16)
        nc.sync.dma_start(out=wo_sb[:], in_=wo[:])

        tiles_q = []

        def admit():
            while blk_unlocked:
                b, jb = blk_unlocked.pop(0)
                st = stage_block(b, jb)
                tiles_q.extend(
                    (st, b, 4 * jb + r, n)
                    for r in range(4)
                    for n in range(H // 512)
                )

        bchunks = 0
        emitted = 0
        # drain rest of b=0 attention, o_proj of unlocked b=0 blocks mixed in
        while chunks < B0_CHUNKS:
            try:
                next(ag)
            except StopIteration:
                break
            chunks += 1
            bchunks += 1
            admit()
            if bchunks > 12:
                while emitted < bchunks - 12 and len(tiles_q) > 4:
                    emit_oproj(*tiles_q.pop(0))
                    emitted += 1
        bg_done = False
        while not bg_done:
            try:
                next(bg)
                bchunks += 1
            except StopIteration:
                bg_done = True
                break
            admit()
            if bchunks > 12:
                while emitted < bchunks - 12 and len(tiles_q) > 4:
                    emit_oproj(*tiles_q.pop(0))
                    emitted += 1
        admit()
        for st_b_mt_n in tiles_q:
            emit_oproj(*st_b_mt_n)

    return nc


_CACHE: dict = {}


def _get_nc(mode: str) -> bass.Bass:
    if mode not in _CACHE:
        _CACHE[mode] = _build(mode)
    return _CACHE[mode]


def _rope_tables():
    inv_freq = 1.0 / (THETA ** (np.arange(0, HD, 2, dtype=np.float32) / HD))
    t = np.arange(S, dtype=np.float32)
    freqs = np.einsum("i,j->ij", t, inv_freq)
    emb = np.concatenate((freqs, freqs), axis=-1)  # [S, HD]
    return np.cos(emb), np.sin(emb)


def kernel(hidden_states, attention_mask, Wq, Wk, Wv, Wo):
    hs = np.asarray(hidden_states, dtype=np.float32)
    mask = np.asarray(attention_mask, dtype=np.float32)[0, 0]
    Wq = np.asarray(Wq, dtype=np.float32)
    Wk = np.asarray(Wk, dtype=np.float32)
    Wv = np.asarray(Wv, dtype=np.float32)
    Wo = np.asarray(Wo, dtype=np.float32)

    causal = np.triu(np.full((S, S), -1e9, dtype=np.float32), k=1)
    if np.array_equal(mask, causal):
        mode = "causal"
    elif not mask.any():
        mode = "zeros"
    else:
        mode = "general"

    # ── host-side prep (blocked layouts) ──
    xt = hs.reshape(T, H).T                       # [H, T] fp32
    # xtb[p, (tb k t)] = xt[k*128+p, tb*512+t]
    xtb = np.ascontiguousarray(
        xt.reshape(KT, 128, NTB, TB).transpose(1, 2, 0, 3).reshape(128, NTB * KT * TB)
    ).astype(BF)
    cos, sin = _rope_tables()                     # [S, HD] fp32
    cost = np.ascontiguousarray(cos.T).astype(BF)   # [HD, S]
    sinp = np.ascontiguousarray(sin.T).astype(BF)
    ones_t = np.ones((128, 128), dtype=BF)
    # rotate_half as matmul: rot = P @ raw with P[i, i+64] = -1, P[i+64, i] = 1
    P = np.zeros((HD, HD), dtype=np.float32)
    for i in range(HD // 2):
        P[i, i + HD // 2] = -1.0
        P[i + HD // 2, i] = 1.0
    ptm = np.ascontiguousarray(P.T).astype(BF)

    common = {"cost": cost, "sinp": sinp, "ones_t": ones_t, "xtb": xtb,
              "pt": ptm}
    if mode == "causal":
        p_idx = np.arange(128)[:, None]
        c_idx = np.arange(512)[None, :]
        md = np.where(p_idx > c_idx, np.float32(0), np.float32(1))
        common["mdiag"] = np.ascontiguousarray(md).astype(BF)
    elif mode == "general":
        common["maskt"] = np.ascontiguousarray(
            np.exp(np.clip(mask.T.astype(np.float64), -80, 11))
        ).astype(BF)

    def wblock(Wslice):  # [OC rows of W, H] -> [128, KT*OC] (k, oc)
        wt = Wslice.T  # [H, OC]
        return np.ascontiguousarray(
            wt.reshape(KT, 128, OC).transpose(1, 0, 2).reshape(128, KT * OC)
        ).astype(BF)

    in_maps = []
    for c in range(NCORES):
        osl = slice(OC * c, OC * (c + 1))
        wot = Wo[:, osl].T  # [OC, H]
        wob = np.ascontiguousarray(
            wot.reshape(HPC, 128, H).transpose(1, 0, 2).reshape(128, HPC * H)
        ).astype(BF)
        in_maps.append(
            dict(
                common,
                wq=wblock(Wq[osl, :]),
                wk=wblock(Wk[osl, :]),
                wv=wblock(Wv[osl, :]),
                wo=wob,
            )
        )

    global _last_in_maps
    _last_in_maps = in_maps
    nc = _get_nc(mode)
    res = run_bass_kernel_spmd(nc, in_maps, list(range(NCORES)))
    out = np.zeros((T, H), dtype=np.float32)
    for c in range(NCORES):
        out += res.results[c]["y"].astype(np.float32)
    return out.reshape(B, S, H)



# revision 11
# speedup vs baseline: 1.0275x; 1.0275x over previous
"""Trainium2 Bass kernel for AliceAttention (dense transformer attention layer).

Sharding: tensor-parallel over the 32 heads -> 4 heads per core across 8
NeuronCores; each core emits a partial o_proj (y_c = ao_c @ Wo[:, cols_c].T)
in fp16 and the 8 partials are summed on the host in fp32.

Layout/engine choices (v3):
  * bf16 everywhere (fp8 blows the 2e-2 error budget: one naked fp8 stage
    measures ~3.8e-2 final rel err).
  * RoPE rotate_half on DVE via partition-swapped half reads against a
    sign-folded sin table (no PE permutation matmul; frees a PSUM bank).
  * Causal diagonal tiles are narrowed: score/exp/AV/acc all operate on
    [r*128:512] only (saves ~6/40 of attention PE work per pair).
  * Scores are computed transposed, scores_T = [t_k, t_q]; exp runs on ACT
    with bias -12 (softmax is shift-invariant; keeps fp16 sums in range).
  * AV matmul trails the score stream by 3 steps to hide exp latency;
    score PSUM pool has 3 banks.
  * Softmax denominators: exp tiles accumulated on DVE in bf16 (two
    accumulators), one ones-matmul per (pair, q-block) reduces over
    partitions; 1/sum via exp(-ln(sum)).
  * Phase overlap: b=1 projections interleave with b=0 attention; o_proj
    tiles interleave with b=1 attention. b=1 attention outputs stay in
    SBUF and feed o_proj directly (no aots round trip).
  * Startup: wq/x t-block-0 loads split into fine chunks emitted in
    consumption order so the first matmul starts within a few us.
"""

import numpy as np
import ml_dtypes
from contextlib import ExitStack

BF = ml_dtypes.bfloat16

import orjson

import concourse.bass as bass
import concourse.mybir as mybir
import concourse.tile as tile
import concourse.bass2jax as bass2jax
from concourse.bass_utils import run_bass_kernel_spmd

# ─────────────────────────────────────────────────────────────────────────
# Walrus in this container rejects instructions carrying more semaphore
# waits than their ISA struct can hold. Split excess waits into wait-only
# EventSemaphore instructions on the same engine — semantically identical.
# ─────────────────────────────────────────────────────────────────────────
_WAIT_CAP = {"EventSemaphore": 2}
_DEFAULT_WAIT_CAP = 1


def _legalize_bir_waits(bir_bytes: bytes) -> bytes:
    d = orjson.loads(bir_bytes)
    changed = False
    for fn in d.get("functions", []):
        for blk in fn.get("blocks", []):
            insts = blk.get("instructions")
            if not insts:
                continue
            out = []
            for inst in insts:
                si = inst.get("sync_info")
                waits = (si or {}).get("on_wait") or []
                cap = _WAIT_CAP.get(inst.get("opcode"), _DEFAULT_WAIT_CAP)
                if len(waits) > cap:
                    excess, keep = waits[:-cap], waits[-cap:]
                    for i in range(0, len(excess), 2):
                        out.append(
                            {
                                "debug": inst.get("debug"),
                                "engine": inst["engine"],
                                "ins": [],
                                "outs": [],
                                "name": f"{inst['name']}_xw{i}",
                                "opcode": "EventSemaphore",
                                "sync_info": {
                                    "on_update": [],
                                    "on_wait": excess[i : i + 2],
                                },
                            }
                        )
                    si["on_wait"] = keep
                    changed = True
                out.append(inst)
            blk["instructions"] = out
    return orjson.dumps(d) if changed else bir_bytes


if not getattr(bass2jax, "_wait_legalize_patched", False):
    _orig_compile_bir_kernel = bass2jax.compile_bir_kernel

    def _patched_compile_bir_kernel(ant_bir_str, compile_dir_path, **kw):
        return _orig_compile_bir_kernel(
            _legalize_bir_waits(ant_bir_str), compile_dir_path, **kw
        )

    bass2jax.compile_bir_kernel = _patched_compile_bir_kernel
    bass2jax._wait_legalize_patched = True

# ─────────────────────────────────────────────────────────────────────────
# Problem constants (hardcoded per contract)
# ─────────────────────────────────────────────────────────────────────────
B, S, H, NH, HD = 2, 2048, 4096, 32, 128
THETA = 10000.0
NCORES = 8
HPC = NH // NCORES          # heads per core = 4
OC = HPC * HD               # output cols per core = 512
T = B * S                   # 4096 tokens
KT = H // 128               # 32 contraction tiles for projections
KTQ = KT // 4               # 8 tiles per x quarter-block
TB = 512                    # t-block width in phase A
NTB = T // TB               # 8 t-blocks (0-3 = b0, 4-7 = b1)
NQ = S // 512               # 4 query blocks per pair
NK = S // 128               # 16 key tiles per pair
SCALE = 1.0 / float(np.sqrt(HD))
EXP_BIAS = -12.0            # softmax shift; cancels in the normalization
AV_DELAY = 3                # score->AV lag in steps (hides exp latency)

F32 = mybir.dt.float32
F16 = mybir.dt.bfloat16  # device 16-bit dtype: bf16 (fp16 runs 0.8x on PE)
EXPF = mybir.ActivationFunctionType.Exp
LNF = mybir.ActivationFunctionType.Ln


def _kept(j, mode):
    if mode == "causal":
        return list(range(min(NK, 4 * j + 4)))
    return list(range(NK))


def _build(mode: str) -> bass.Bass:
    """mode: 'causal' (skip masked tiles, narrowed diagonal),
    'zeros' (no mask, all tiles), 'general' (stream fp16 mask tiles)."""
    nc = bass.Bass()

    # blocked layouts (host-prepared):
    #   xtb [128, tb*KT*TB]   x[(k*128+p), tb*512+t] at col ((tb*KT+k)*TB+t)
    #   wq/wk/wv [128, KT*OC] W.T[(k*128+p), oc] at col (k*OC+oc)
    #   wo [128, HPC*H]       Wo[:,osl].T[(hl*128+p), n] at col (hl*H+n)
    xtb = nc.declare_dram_parameter("xtb", [128, NTB * KT * TB], F16, isOutput=False)
    wq = nc.declare_dram_parameter("wq", [128, KT * OC], F16, isOutput=False)
    wk = nc.declare_dram_parameter("wk", [128, KT * OC], F16, isOutput=False)
    wv = nc.declare_dram_parameter("wv", [128, KT * OC], F16, isOutput=False)
    wo = nc.declare_dram_parameter("wo", [128, HPC * H], F16, isOutput=False)
    cost = nc.declare_dram_parameter("cost", [HD, S], F16, isOutput=False)
    sinp = nc.declare_dram_parameter("sinp", [HD, S], F16, isOutput=False)  # sign-folded
    ones_t = nc.declare_dram_parameter("ones_t", [128, 128], F16, isOutput=False)
    if mode == "causal":
        mdiag = nc.declare_dram_parameter("mdiag", [128, 128], F16, isOutput=False)
    elif mode == "general":
        maskt = nc.declare_dram_parameter("maskt", [S, S], F16, isOutput=False)
    y = nc.declare_dram_parameter("y", [T, H], F16, isOutput=True)

    # DRAM scratch: roped qT/kT per head [128, T] rows hl*128+p, col = t;
    # v blocked the same way: row hl*128+p, col = global k-tile*128 + d
    qts = nc.dram_tensor("qts", [OC, T], F16)
    kts = nc.dram_tensor("kts", [OC, T], F16)
    vts = nc.dram_tensor("vts", [OC, T], F16)
    aots = nc.dram_tensor("aots", [OC, S], F16)  # b=0 only; b=1 stays in SBUF

    # register the exp bias constant (activation() needs a const AP for it)
    _bias_t = nc.alloc_sbuf_tensor("const-exp-bias", [128, 1], F32)
    nc.gpsimd.memset(_bias_t.ap(), EXP_BIAS)
    nc.const_aps.aps[(F32, EXP_BIAS)] = _bias_t.ap()

    with tile.TileContext(nc) as tc, ExitStack() as octx:
        const_pool = octx.enter_context(tc.tile_pool(name="const", bufs=1))
        qk_pool = octx.enter_context(tc.tile_pool(name="qkv_pair", bufs=2))
        exp_pool = octx.enter_context(tc.tile_pool(name="exp", bufs=AV_DELAY + 1))
        acc_pool = octx.enter_context(tc.tile_pool(name="acc", bufs=2))
        nrm_pool = octx.enter_context(tc.tile_pool(name="nrm", bufs=1))
        asp_pool = octx.enter_context(tc.tile_pool(name="aosp", bufs=2))
        st_pool = None   # created in R3, after the A-phase pools release
        spb_pool = None  # created in R3, after the A-phase pools release
        if mode == "general":
            mt_pool = octx.enter_context(tc.tile_pool(name="mtile", bufs=4))
        sc_pool = octx.enter_context(tc.tile_pool(name="psSc", bufs=3, space="PSUM"))
        av_pool = octx.enter_context(tc.tile_pool(name="psAv", bufs=2, space="PSUM"))
        sm_pool = octx.enter_context(tc.tile_pool(name="psSum", bufs=1, space="PSUM"))

        blk_unlocked = []  # (b, j) blocks whose o_proj inputs are ready
        spb = {}           # (j, hl) -> SBUF sp tile for b=1

        # ═══════ attention chain (pair-major), as a generator ═══════
        def load_pair(b, hl):
            osl = slice(hl * 128, (hl + 1) * 128)
            bsl = slice(b * S, (b + 1) * S)
            qT = qk_pool.tile([HD, S], F16, tag="qT", name=f"qT{b}{hl}")
            kT = qk_pool.tile([HD, S], F16, tag="kT", name=f"kT{b}{hl}")
            v = qk_pool.tile([128, NK * HD], F16, tag="v", name=f"v{b}{hl}")
            nc.sync.dma_start(out=kT[:], in_=kts[osl, bsl])
            nc.sync.dma_start(out=qT[:], in_=qts[osl, bsl])
            nc.sync.dma_start(out=v[:], in_=vts[osl, bsl])
            return qT, kT, v

        def attn_chain(order):
            """Pair-major attention over the given (b, hl) pairs. First yield
            emits only the first two pair loads (prime it early; all spills
            for these batches must already be emitted - DRAM RAW deps are
            emission-ordered); later yields are one ki-step each."""
            pending = [load_pair(*order[0]), load_pair(*order[1])]
            yield  # prime point: loads emitted, no compute yet
            for pi, (b, hl) in enumerate(order):
                qT, kT, v = pending.pop(0)
                if pi + 2 < len(order):
                    pending.append(load_pair(*order[pi + 2]))
                steps = [(j, ki) for j in range(NQ) for ki in _kept(j, mode)]
                pend = []  # delayed AV steps: (j, ki, off, exp_sb, first, last)
                avps = {}
                accs = {}

                def finish(stp, b=b, hl=hl, v=v, avps=avps, accs=accs):
                    j, ki, off, exp_sb, first, last = stp
                    nc.tensor.matmul(
                        avps[j][:, off:],
                        v[:, ki * HD : (ki + 1) * HD],
                        exp_sb[:, off:],
                        start=first,
                        stop=last,
                        skip_group_check=(mode == "causal"),
                    )
                    if last:
                        a0, a1, cnt = accs[j]
                        if cnt > 1:
                            nc.vector.tensor_add(a0[:], a0[:], a1[:])
                        sm_ps = sm_pool.tile([128, 512], F32, tag="sm")
                        nc.tensor.matmul(
                            sm_ps[:], ones_sb[:], a0[:], start=True, stop=True
                        )
                        ln_sb = nrm_pool.tile([128, 512], F32, tag="lnv")
                        nc.scalar.activation(ln_sb[:], sm_ps[:], LNF)
                        rc = nrm_pool.tile([128, 512], F32, tag="rc")
                        nc.scalar.activation(rc[:], ln_sb[:], EXPF, scale=-1.0)
                        if b == 0:
                            sp = asp_pool.tile([128, 512], F16, tag="aosp", name="sp")
                        else:
                            sp = spb_pool.tile(
                                [128, 512], F16, tag=f"sp{hl}_{j}", name=f"sp{hl}{j}"
                            )
                        nc.vector.tensor_mul(sp[:], avps[j][:], rc[:])
                        if b == 0:
                            nc.sync.dma_start(
                                out=aots[
                                    hl * 128 : (hl + 1) * 128,
                                    j * 512 : (j + 1) * 512,
                                ],
                                in_=sp[:],
                            )
                        else:
                            spb[(j, hl)] = sp
                        if hl == HPC - 1:
                            blk_unlocked.append((b, j))

                for j, ki in steps:
                    kept = _kept(j, mode)
                    first, last = ki == kept[0], ki == kept[-1]
                    r = ki - 4 * j
                    off = r * 128 if (mode == "causal" and r > 0) else 0
                    if first:
                        avps[j] = av_pool.tile(
                            [128, 512], F32, tag="av", name=f"av{pi}_{j}"
                        )
                        accs[j] = [None, None, 0]
                    sc_ps = sc_pool.tile([128, 512], F32, tag="sc")
                    nc.tensor.matmul(
                        sc_ps[:, off:],
                        kT[:, ki * 128 : (ki + 1) * 128],
                        qT[:, j * 512 + off : (j + 1) * 512],
                        start=True,
                        stop=True,
                    )
                    exp_sb = exp_pool.tile([128, 512], F16, tag="exp")
                    nc.scalar.activation(
                        exp_sb[:, off:], sc_ps[:, off:], EXPF, scale=SCALE, bias=EXP_BIAS
                    )
                    if mode == "causal" and r >= 0 and ki >= 4 * j:
                        nc.vector.tensor_mul(
                            exp_sb[:, off : off + 128],
                            exp_sb[:, off : off + 128],
                            md_sb[:],
                        )
                    elif mode == "general":
                        m_sb = mt_pool.tile([128, 512], F16, tag="mt")
                        nc.sync.dma_start(
                            out=m_sb[:],
                            in_=maskt[
                                ki * 128 : (ki + 1) * 128, j * 512 : (j + 1) * 512
                            ],
                        )
                        nc.vector.tensor_mul(exp_sb[:], exp_sb[:], m_sb[:])
                    # bf16 two-accumulator chain for the softmax denominator
                    a = accs[j]
                    w = a[2] % 2
                    if a[2] < 2:
                        t_acc = acc_pool.tile(
                            [128, 512], F16, tag=f"acc{w}", name=f"acc{pi}_{j}_{w}"
                        )
                        if off > 0:
                            nc.vector.memset(t_acc[:, :off], 0.0)
                        nc.vector.tensor_copy(t_acc[:, off:], exp_sb[:, off:])
                        a[w] = t_acc
                    else:
                        nc.vector.tensor_add(
                            a[w][:, off:], a[w][:, off:], exp_sb[:, off:]
                        )
                    a[2] += 1
                    pend.append((j, ki, off, exp_sb, first, last))
                    if len(pend) > AV_DELAY:
                        finish(pend.pop(0))
                    yield
                for stp in pend:
                    finish(stp)
                yield

        # ═══════ o_proj tile (b=0 staged from aots; b=1 direct SBUF) ═══════
        oproj_count = [0]

        def stage_block(jb):
            st = {}
            for hl in range(HPC):
                t_st = st_pool.tile(
                    [128, 512], F16, tag=f"st{hl}", name=f"st0{jb}{hl}"
                )
                nc.sync.dma_start(
                    out=t_st[:],
                    in_=aots[
                        hl * 128 : (hl + 1) * 128,
                        jb * 512 : (jb + 1) * 512,
                    ],
                )
                st[hl] = t_st
            return st

        def emit_oproj(st, b, mt, n):
            msl = slice(b * S + mt * 128, b * S + (mt + 1) * 128)
            ps = yp_pool.tile([128, 512], F32, tag="y")
            for hl in range(HPC):
                nc.tensor.matmul(
                    ps[:],
                    st[hl][:, (mt % 4) * 128 : (mt % 4 + 1) * 128],
                    wo_sb[:, hl * H + n * 512 : hl * H + (n + 1) * 512],
                    start=(hl == 0),
                    stop=(hl == HPC - 1),
                )
            y_sb = yo_pool.tile([128, 512], F16, tag="ysb")
            if oproj_count[0] % 2 == 0:
                nc.scalar.copy(y_sb[:], ps[:])
            else:
                nc.vector.tensor_copy(y_sb[:], ps[:])
            oproj_count[0] += 1
            nc.sync.dma_start(out=y[msl, n * 512 : (n + 1) * 512], in_=y_sb[:])

        # ═══════ Phase A (+R2): projections + RoPE, spill to DRAM ═══════
        with ExitStack() as actx:
            w_pool = actx.enter_context(tc.tile_pool(name="wqk", bufs=1))
            x_pool = actx.enter_context(tc.tile_pool(name="xblk", bufs=2))
            ev_pool = actx.enter_context(tc.tile_pool(name="evac", bufs=2))
            rp_pool = actx.enter_context(tc.tile_pool(name="rope", bufs=1))
            vh_pool = actx.enter_context(tc.tile_pool(name="vhl", bufs=1))
            ps_pool = actx.enter_context(
                tc.tile_pool(name="psA", bufs=2, space="PSUM")
            )

            wq_sb = w_pool.tile([128, KT * OC], F16, tag="wq")
            wk_sb = w_pool.tile([128, KT * OC], F16, tag="wk")
            wv_sb = w_pool.tile([128, KT * OC], F16, tag="wv")

            def load_x_quarter(tb, qi, nchunks=1):
                xh = x_pool.tile(
                    [128, KTQ * TB],
                    F16,
                    tag=f"xq{qi}",
                    name=f"x{tb}{qi}",
                    bufs=2 if qi < 2 else 1,
                )
                base = (tb * KT + qi * KTQ) * TB
                csz = KTQ * TB // nchunks
                for c in range(nchunks):
                    nc.sync.dma_start(
                        out=xh[:, c * csz : (c + 1) * csz],
                        in_=xtb[:, base + c * csz : base + (c + 1) * csz],
                    )
                return xh

            # start-ramp: wq + x(tb0) in fine chunks, consumption-ordered.
            # The first matmul needs only wq k-chunk 0 and x chunk 0, so the
            # leading loads are small; later loads coarsen.
            def wload(w_sb, wsrc, klo, khi):
                nc.sync.dma_start(
                    out=w_sb[:, klo * OC : khi * OC], in_=wsrc[:, klo * OC : khi * OC]
                )

            # x quarter 0 of tb0 in 4 chunks of 2 k (256 KB each)
            x0q = []
            wload(wq_sb, wq, 0, 1)
            x0q.append(load_x_quarter(0, 0, nchunks=4))
            wload(wq_sb, wq, 1, 4)
            wload(wq_sb, wq, 4, 8)
            x0q.append(load_x_quarter(0, 1, nchunks=2))
            wload(wq_sb, wq, 8, 16)
            cos_sb = const_pool.tile([HD, S], F16)
            nc.sync.dma_start(out=cos_sb[:], in_=cost[:])
            x0q.append(load_x_quarter(0, 2, nchunks=2))
            sin_sb = const_pool.tile([HD, S], F16)
            nc.sync.dma_start(out=sin_sb[:], in_=sinp[:])
            wload(wq_sb, wq, 16, 24)
            x0q.append(load_x_quarter(0, 3, nchunks=2))
            wload(wq_sb, wq, 24, 32)
            for c in range(4):
                wload(wk_sb, wk, c * 8, (c + 1) * 8)

            ones_sb = const_pool.tile([128, 128], F16)
            nc.sync.dma_start(out=ones_sb[:], in_=ones_t[:])
            if mode == "causal":
                md_sb = const_pool.tile([128, 128], F16)
                nc.sync.dma_start(out=md_sb[:], in_=mdiag[:])

            for c in range(4):
                wload(wv_sb, wv, c * 8, (c + 1) * 8)

            def emit_tb(tb, xq=None):
                """Projections for t-block tb; yields after each of 13 groups."""
                if xq is None:
                    xq = [load_x_quarter(tb, qi) for qi in range(4)]
                xh = xq
                tsl = slice((tb % 4) * TB, (tb % 4 + 1) * TB)  # cos/sin cols
                gsl = slice(tb * TB, (tb + 1) * TB)            # global t cols
                def _qk_groups():
                    for which, w_sb, spill in (("q", wq_sb, qts), ("k", wk_sb, kts)):
                        for ot in range(HPC):
                            ps = ps_pool.tile([128, TB], F32, tag="proj")
                            for k in range(KT):
                                nc.tensor.matmul(
                                    ps[:],
                                    w_sb[:, k * OC + ot * 128 : k * OC + (ot + 1) * 128],
                                    xh[k // KTQ][:, (k % KTQ) * TB : (k % KTQ + 1) * TB],
                                    start=(k == 0),
                                    stop=(k == KT - 1),
                                )
                            raw = ev_pool.tile([128, TB], F16, tag="raw")
                            nc.scalar.copy(raw[:], ps[:])
                            # RoPE on DVE: t1 = raw*cos; t2 = rot_half(raw)*sin
                            # via partition-swapped reads of a sign-folded sin.
                            # The swapped reads come from PSUM (ps) — the DVE
                            # same-base-partition rule only binds SB+SB pairs.
                            t1 = rp_pool.tile([128, TB], F16, tag="t1")
                            nc.vector.tensor_mul(t1[:], raw[:], cos_sb[:, tsl])
                            t2 = rp_pool.tile([128, TB], F16, tag="t2")
                            nc.vector.tensor_mul(
                                t2[0:64, :], ps[64:128, :], sin_sb[0:64, tsl]
                            )
                            nc.vector.tensor_mul(
                                t2[64:128, :], ps[0:64, :], sin_sb[64:128, tsl]
                            )
                            nc.vector.tensor_add(raw[:], t1[:], t2[:])
                            nc.sync.dma_start(
                                out=spill[ot * 128 : (ot + 1) * 128, gsl], in_=raw[:]
                            )
                            yield
                def _v_groups():
                    vhl = [
                        vh_pool.tile([128, TB], F16, tag=f"vhl{hl}", name=f"vhl{hl}")
                        for hl in range(HPC)
                    ]
                    for mt in range(TB // 128):
                        ps = ps_pool.tile([128, OC], F32, tag="proj", name="psv")
                        for k in range(KT):
                            nc.tensor.matmul(
                                ps[:],
                                xh[k // KTQ][
                                    :, (k % KTQ) * TB + mt * 128 : (k % KTQ) * TB + (mt + 1) * 128
                                ],
                                wv_sb[:, k * OC : (k + 1) * OC],
                                start=(k == 0),
                                stop=(k == KT - 1),
                            )
                        for hl in range(HPC):
                            nc.scalar.copy(
                                vhl[hl][:, mt * 128 : (mt + 1) * 128],
                                ps[:, hl * 128 : (hl + 1) * 128],
                            )
                        yield
                    for hl in range(HPC):
                        nc.sync.dma_start(
                            out=vts[hl * 128 : (hl + 1) * 128, gsl], in_=vhl[hl][:]
                        )
                segs = (_v_groups, _qk_groups) if tb == 7 else (_qk_groups, _v_groups)
                for _seg in segs:
                    yield from _seg()

            PAIR_CHUNKS = sum(len(_kept(j, mode)) for j in range(NQ)) + 1
            B0_CHUNKS = HPC * PAIR_CHUNKS
            ag = attn_chain([(0, hl) for hl in range(HPC)])
            chunks = 0

            def pump(n_target):
                nonlocal chunks
                while chunks < n_target:
                    try:
                        next(ag)
                    except StopIteration:
                        return False
                    chunks += 1
                return True

            # A1: b=0 projections, dense; prime pair loads at the tail
            for tb in range(4):
                g = emit_tb(tb, x0q if tb == 0 else None)
                for _ in g:
                    pass
            next(ag)  # prime: emits first two pair loads only

            # R2: b=1 projections interleaved with b=0 attention
            groups = 0
            for tb in range(4, 8):
                for _ in emit_tb(tb):
                    groups += 1
                    pump(min((B0_CHUNKS * groups) // (4 * 12) + 1, B0_CHUNKS))
            # all b=1 spills are emitted now: safe to prime the b=1 chain;
            # its pair-0/1 loads overlap the b=0 attention drain below
            bg = attn_chain([(1, hl) for hl in range(HPC)])
            next(bg)

        # ═══════ R3: o_proj interleaved with remaining attention ═══════
        wo_pool = octx.enter_context(tc.tile_pool(name="wo", bufs=1))
        yo_pool = octx.enter_context(tc.tile_pool(name="yout", bufs=3))
        st_pool = octx.enter_context(tc.tile_pool(name="stage", bufs=3))
        spb_pool = octx.enter_context(tc.tile_pool(name="spb", bufs=1))
        yp_pool = octx.enter_context(tc.tile_pool(name="psY", bufs=2, space="PSUM"))

        wo_sb = wo_pool.tile([128, HPC * H], F16)
        nc.sync.dma_start(out=wo_sb[:], in_=wo[:])

        tiles_q = []

        def admit():
            while blk_unlocked:
                b, jb = blk_unlocked.pop(0)
                if b == 0:
                    st = stage_block(jb)
                else:
                    st = {hl: spb[(jb, hl)] for hl in range(HPC)}
                tiles_q.extend(
                    (st, b, 4 * jb + r, n)
                    for r in range(4)
                    for n in range(H // 512)
                )

        bchunks = 0
        emitted = 0
        # drain rest of b=0 attention, o_proj of unlocked b=0 blocks mixed in
        while chunks < B0_CHUNKS:
            try:
                next(ag)
            except StopIteration:
                break
            chunks += 1
            bchunks += 1
            admit()
            if bchunks > 12:
                while emitted < bchunks - 12 and len(tiles_q) > 4:
                    emit_oproj(*tiles_q.pop(0))
                    emitted += 1
        bg_done = False
        while not bg_done:
            try:
                next(bg)
                bchunks += 1
            except StopIteration:
                bg_done = True
                break
            admit()
            if bchunks > 12:
                while emitted < bchunks - 12 and len(tiles_q) > 4:
                    emit_oproj(*tiles_q.pop(0))
                    emitted += 1
        admit()
        for st_b_mt_n in tiles_q:
            emit_oproj(*st_b_mt_n)

    return nc


_CACHE: dict = {}


def _get_nc(mode: str) -> bass.Bass:
    if mode not in _CACHE:
        _CACHE[mode] = _build(mode)
    return _CACHE[mode]


def _rope_tables():
    inv_freq = 1.0 / (THETA ** (np.arange(0, HD, 2, dtype=np.float32) / HD))
    t = np.arange(S, dtype=np.float32)
    freqs = np.einsum("i,j->ij", t, inv_freq)
    emb = np.concatenate((freqs, freqs), axis=-1)  # [S, HD]
    return np.cos(emb), np.sin(emb)


def kernel(hidden_states, attention_mask, Wq, Wk, Wv, Wo):
    hs = np.asarray(hidden_states, dtype=np.float32)
    mask = np.asarray(attention_mask, dtype=np.float32)[0, 0]
    Wq = np.asarray(Wq, dtype=np.float32)
    Wk = np.asarray(Wk, dtype=np.float32)
    Wv = np.asarray(Wv, dtype=np.float32)
    Wo = np.asarray(Wo, dtype=np.float32)

    causal = np.triu(np.full((S, S), -1e9, dtype=np.float32), k=1)
    if np.array_equal(mask, causal):
        mode = "causal"
    elif not mask.any():
        mode = "zeros"
    else:
        mode = "general"

    # ── host-side prep (blocked layouts) ──
    xt = hs.reshape(T, H).T                       # [H, T] fp32
    # xtb[p, (tb k t)] = xt[k*128+p, tb*512+t]
    xtb = np.ascontiguousarray(
        xt.reshape(KT, 128, NTB, TB).transpose(1, 2, 0, 3).reshape(128, NTB * KT * TB)
    ).astype(BF)
    cos, sin = _rope_tables()                     # [S, HD] fp32
    cost = np.ascontiguousarray(cos.T).astype(BF)   # [HD, S]
    sinf = sin.T.copy()                             # [HD, S], sign-folded
    sinf[: HD // 2] *= -1.0
    sinp = np.ascontiguousarray(sinf).astype(BF)
    ones_t = np.ones((128, 128), dtype=BF)

    common = {"cost": cost, "sinp": sinp, "ones_t": ones_t, "xtb": xtb}
    if mode == "causal":
        p_idx = np.arange(128)[:, None]
        c_idx = np.arange(128)[None, :]
        md = np.where(p_idx > c_idx, np.float32(0), np.float32(1))
        common["mdiag"] = np.ascontiguousarray(md).astype(BF)
    elif mode == "general":
        common["maskt"] = np.ascontiguousarray(
            np.exp(np.clip(mask.T.astype(np.float64), -80, 11))
        ).astype(BF)

    def wblock(Wslice):  # [OC rows of W, H] -> [128, KT*OC] (k, oc)
        wt = Wslice.T  # [H, OC]
        return np.ascontiguousarray(
            wt.reshape(KT, 128, OC).transpose(1, 0, 2).reshape(128, KT * OC)
        ).astype(BF)

    in_maps = []
    for c in range(NCORES):
        osl = slice(OC * c, OC * (c + 1))
        wot = Wo[:, osl].T  # [OC, H]
        wob = np.ascontiguousarray(
            wot.reshape(HPC, 128, H).transpose(1, 0, 2).reshape(128, HPC * H)
        ).astype(BF)
        in_maps.append(
            dict(
                common,
                wq=wblock(Wq[osl, :]),
                wk=wblock(Wk[osl, :]),
                wv=wblock(Wv[osl, :]),
                wo=wob,
            )
        )

    global _last_in_maps
    _last_in_maps = in_maps
    nc = _get_nc(mode)
    res = run_bass_kernel_spmd(nc, in_maps, list(range(NCORES)))
    out = np.zeros((T, H), dtype=np.float32)
    for c in range(NCORES):
        out += res.results[c]["y"].astype(np.float32)
    return out.reshape(B, S, H)


# revision 20
# speedup vs baseline: 1.0749x; 1.0462x over previous
"""Trainium2 Bass kernel for AliceAttention (dense transformer attention layer).

Sharding: tensor-parallel over the 32 heads -> 4 heads per core across 8
NeuronCores; each core emits a partial o_proj (y_c = ao_c @ Wo[:, cols_c].T)
in fp16 and the 8 partials are summed on the host in fp32.

Layout/engine choices (v3):
  * bf16 everywhere (fp8 blows the 2e-2 error budget: one naked fp8 stage
    measures ~3.8e-2 final rel err).
  * RoPE rotate_half on DVE via partition-swapped half reads against a
    sign-folded sin table (no PE permutation matmul; frees a PSUM bank).
  * Causal diagonal tiles are narrowed: score/exp/AV/acc all operate on
    [r*128:512] only (saves ~6/40 of attention PE work per pair).
  * Scores are computed transposed, scores_T = [t_k, t_q]; exp runs on ACT
    with bias -12 (softmax is shift-invariant; keeps fp16 sums in range).
  * AV matmul trails the score stream by 3 steps to hide exp latency;
    score PSUM pool has 3 banks.
  * Softmax denominators: exp tiles accumulated on DVE in bf16 (two
    accumulators), one ones-matmul per (pair, q-block) reduces over
    partitions; 1/sum via exp(-ln(sum)).
  * Phase overlap: b=1 projections interleave with b=0 attention; o_proj
    tiles interleave with b=1 attention. b=1 attention outputs stay in
    SBUF and feed o_proj directly (no aots round trip).
  * Startup: wq/x t-block-0 loads split into fine chunks emitted in
    consumption order so the first matmul starts within a few us.
"""

import numpy as np
import ml_dtypes
from contextlib import ExitStack

BF = ml_dtypes.bfloat16

import orjson

import concourse.bass as bass
import concourse.mybir as mybir
import concourse.tile as tile
import concourse.bass2jax as bass2jax
from concourse.bass_utils import run_bass_kernel_spmd

# ─────────────────────────────────────────────────────────────────────────
# Walrus in this container rejects instructions carrying more semaphore
# waits than their ISA struct can hold. Split excess waits into wait-only
# EventSemaphore instructions on the same engine — semantically identical.
# ─────────────────────────────────────────────────────────────────────────
_WAIT_CAP = {"EventSemaphore": 2}
_DEFAULT_WAIT_CAP = 1


def _legalize_bir_waits(bir_bytes: bytes) -> bytes:
    d = orjson.loads(bir_bytes)
    changed = False
    for fn in d.get("functions", []):
        for blk in fn.get("blocks", []):
            insts = blk.get("instructions")
            if not insts:
                continue
            out = []
            for inst in insts:
                si = inst.get("sync_info")
                waits = (si or {}).get("on_wait") or []
                cap = _WAIT_CAP.get(inst.get("opcode"), _DEFAULT_WAIT_CAP)
                if len(waits) > cap:
                    excess, keep = waits[:-cap], waits[-cap:]
                    for i in range(0, len(excess), 2):
                        out.append(
                            {
                                "debug": inst.get("debug"),
                                "engine": inst["engine"],
                                "ins": [],
                                "outs": [],
                                "name": f"{inst['name']}_xw{i}",
                                "opcode": "EventSemaphore",
                                "sync_info": {
                                    "on_update": [],
                                    "on_wait": excess[i : i + 2],
                                },
                            }
                        )
                    si["on_wait"] = keep
                    changed = True
                out.append(inst)
            blk["instructions"] = out
    return orjson.dumps(d) if changed else bir_bytes


if not getattr(bass2jax, "_wait_legalize_patched", False):
    _orig_compile_bir_kernel = bass2jax.compile_bir_kernel

    def _patched_compile_bir_kernel(ant_bir_str, compile_dir_path, **kw):
        return _orig_compile_bir_kernel(
            _legalize_bir_waits(ant_bir_str), compile_dir_path, **kw
        )

    bass2jax.compile_bir_kernel = _patched_compile_bir_kernel
    bass2jax._wait_legalize_patched = True

# ─────────────────────────────────────────────────────────────────────────
# Problem constants (hardcoded per contract)
# ─────────────────────────────────────────────────────────────────────────
B, S, H, NH, HD = 2, 2048, 4096, 32, 128
THETA = 10000.0
NCORES = 8
HPC = NH // NCORES          # heads per core = 4
OC = HPC * HD               # output cols per core = 512
T = B * S                   # 4096 tokens
KT = H // 128               # 32 contraction tiles for projections
KTQ = KT // 4               # 8 tiles per x quarter-block
TB = 512                    # t-block width in phase A
NTB = T // TB               # 8 t-blocks (0-3 = b0, 4-7 = b1)
NQ = S // 512               # 4 query blocks per pair
NK = S // 128               # 16 key tiles per pair
SCALE = 1.0 / float(np.sqrt(HD))
EXP_BIAS = -12.0            # softmax shift; cancels in the normalization
AV_DELAY = 4                # score->AV lag in steps (hides exp latency)

F32 = mybir.dt.float32
F16 = mybir.dt.bfloat16  # device 16-bit dtype: bf16 (fp16 runs 0.8x on PE)
EXPF = mybir.ActivationFunctionType.Exp
LNF = mybir.ActivationFunctionType.Ln


def _kept(j, mode):
    if mode == "causal":
        return list(range(min(NK, 4 * j + 4)))
    return list(range(NK))


def _build(mode: str) -> bass.Bass:
    """mode: 'causal' (skip masked tiles, narrowed diagonal),
    'zeros' (no mask, all tiles), 'general' (stream fp16 mask tiles)."""
    nc = bass.Bass()

    # blocked layouts (host-prepared):
    #   xtb [128, tb*KT*TB]   x[(k*128+p), tb*512+t] at col ((tb*KT+k)*TB+t)
    #   wq/wk/wv [128, KT*OC] W.T[(k*128+p), oc] at col (k*OC+oc)
    #   wo [128, HPC*H]       Wo[:,osl].T[(hl*128+p), n] at col (hl*H+n)
    xtb = nc.declare_dram_parameter("xtb", [128, NTB * KT * TB], F16, isOutput=False)
    wq = nc.declare_dram_parameter("wq", [128, KT * OC], F16, isOutput=False)
    wk = nc.declare_dram_parameter("wk", [128, KT * OC], F16, isOutput=False)
    wv = nc.declare_dram_parameter("wv", [128, KT * OC], F16, isOutput=False)
    wo = nc.declare_dram_parameter("wo", [128, HPC * H], F16, isOutput=False)
    cost = nc.declare_dram_parameter("cost", [HD, S], F16, isOutput=False)
    sinp = nc.declare_dram_parameter("sinp", [HD, S], F16, isOutput=False)  # sign-folded
    ones_t = nc.declare_dram_parameter("ones_t", [128, 128], F16, isOutput=False)
    if mode == "causal":
        mdiag = nc.declare_dram_parameter("mdiag", [128, 128], F16, isOutput=False)
    elif mode == "general":
        maskt = nc.declare_dram_parameter("maskt", [S, S], F16, isOutput=False)
    y = nc.declare_dram_parameter("y", [T, H], F16, isOutput=True)

    # DRAM scratch: roped qT/kT per head [128, T] rows hl*128+p, col = t;
    # v blocked the same way: row hl*128+p, col = global k-tile*128 + d
    qts = nc.dram_tensor("qts", [OC, T], F16)
    kts = nc.dram_tensor("kts", [OC, T], F16)
    vts = nc.dram_tensor("vts", [OC, T], F16)
    aots = nc.dram_tensor("aots", [OC, S], F16)  # b=0 only; b=1 stays in SBUF

    # register the exp bias constant (activation() needs a const AP for it)
    _bias_t = nc.alloc_sbuf_tensor("const-exp-bias", [128, 1], F32)
    nc.gpsimd.memset(_bias_t.ap(), EXP_BIAS)
    nc.const_aps.aps[(F32, EXP_BIAS)] = _bias_t.ap()

    with tile.TileContext(nc) as tc, ExitStack() as octx:
        const_pool = octx.enter_context(tc.tile_pool(name="const", bufs=1))
        qk_pool = octx.enter_context(tc.tile_pool(name="qkv_pair", bufs=2))
        exp_pool = octx.enter_context(tc.tile_pool(name="exp", bufs=AV_DELAY + 1))
        acc_pool = octx.enter_context(tc.tile_pool(name="acc", bufs=2))
        nrm_pool = octx.enter_context(tc.tile_pool(name="nrm", bufs=1))
        asp_pool = octx.enter_context(tc.tile_pool(name="aosp", bufs=2))
        st_pool = None   # created in R3, after the A-phase pools release
        spb_pool = None  # created in R3, after the A-phase pools release
        if mode == "general":
            mt_pool = octx.enter_context(tc.tile_pool(name="mtile", bufs=4))
        sc_pool = octx.enter_context(tc.tile_pool(name="psSc", bufs=2, space="PSUM"))
        av_pool = octx.enter_context(tc.tile_pool(name="psAv", bufs=2, space="PSUM"))
        sm_pool = octx.enter_context(tc.tile_pool(name="psSum", bufs=1, space="PSUM"))

        blk_unlocked = []  # (b, j) blocks whose o_proj inputs are ready
        spb = {}           # (j, hl) -> SBUF sp tile for b=1

        # ═══════ attention chain (pair-major), as a generator ═══════
        def load_pair(b, hl):
            osl = slice(hl * 128, (hl + 1) * 128)
            bsl = slice(b * S, (b + 1) * S)
            qT = qk_pool.tile([HD, S], F16, tag="qT", name=f"qT{b}{hl}")
            kT = qk_pool.tile([HD, S], F16, tag="kT", name=f"kT{b}{hl}")
            v = qk_pool.tile([128, NK * HD], F16, tag="v", name=f"v{b}{hl}")
            nc.sync.dma_start(out=kT[:], in_=kts[osl, bsl])
            nc.sync.dma_start(out=qT[:], in_=qts[osl, bsl])
            nc.sync.dma_start(out=v[:], in_=vts[osl, bsl])
            return qT, kT, v

        def attn_chain(order):
            """Pair-major attention over the given (b, hl) pairs. First yield
            emits only the first two pair loads (prime it early; all spills
            for these batches must already be emitted - DRAM RAW deps are
            emission-ordered); later yields are one ki-step each."""
            pending = [load_pair(*order[0]), load_pair(*order[1])]
            yield  # prime point: loads emitted, no compute yet
            for pi, (b, hl) in enumerate(order):
                qT, kT, v = pending.pop(0)
                if pi + 2 < len(order):
                    pending.append(load_pair(*order[pi + 2]))
                steps = [(j, ki) for j in range(NQ) for ki in _kept(j, mode)]
                pend = []  # delayed AV steps: (j, ki, off, exp_sb, first, last)
                avps = {}
                accs = {}

                def finish(stp, b=b, hl=hl, v=v, avps=avps, accs=accs):
                    j, ki, off, exp_sb, first, last = stp
                    nc.tensor.matmul(
                        avps[j][:, off:],
                        v[:, ki * HD : (ki + 1) * HD],
                        exp_sb[:, off:],
                        start=first,
                        stop=last,
                        skip_group_check=(mode == "causal"),
                    )
                    if last:
                        a0, a1, cnt = accs[j]
                        sm_ps = sm_pool.tile([128, 512], F32, tag="sm")
                        nc.tensor.matmul(
                            sm_ps[:], ones_sb[:], a0[:], start=True, stop=True
                        )
                        ln_sb = nrm_pool.tile([128, 512], F32, tag="lnv")
                        nc.scalar.activation(ln_sb[:], sm_ps[:], LNF)
                        rc = nrm_pool.tile([128, 512], F32, tag="rc")
                        nc.scalar.activation(rc[:], ln_sb[:], EXPF, scale=-1.0)
                        if b == 0:
                            sp = asp_pool.tile([128, 512], F16, tag="aosp", name="sp")
                        else:
                            sp = spb_pool.tile(
                                [128, 512], F16, tag=f"sp{hl}_{j}", name=f"sp{hl}{j}"
                            )
                        nc.vector.tensor_mul(sp[:], avps[j][:], rc[:])
                        if b == 0:
                            nc.sync.dma_start(
                                out=aots[
                                    hl * 128 : (hl + 1) * 128,
                                    j * 512 : (j + 1) * 512,
                                ],
                                in_=sp[:],
                            )
                        else:
                            spb[(j, hl)] = sp
                        if hl == HPC - 1:
                            blk_unlocked.append((b, j))

                for j, ki in steps:
                    kept = _kept(j, mode)
                    first, last = ki == kept[0], ki == kept[-1]
                    r = ki - 4 * j
                    off = r * 128 if (mode == "causal" and r > 0) else 0
                    if first:
                        avps[j] = av_pool.tile(
                            [128, 512], F32, tag="av", name=f"av{pi}_{j}"
                        )
                        accs[j] = [None, None, 0]
                    sc_ps = sc_pool.tile([128, 512], F32, tag="sc")
                    nc.tensor.matmul(
                        sc_ps[:, off:],
                        kT[:, ki * 128 : (ki + 1) * 128],
                        qT[:, j * 512 + off : (j + 1) * 512],
                        start=True,
                        stop=True,
                    )
                    exp_sb = exp_pool.tile([128, 512], F16, tag="exp")
                    nc.scalar.activation(
                        exp_sb[:, off:], sc_ps[:, off:], EXPF, scale=SCALE, bias=EXP_BIAS
                    )
                    if mode == "causal" and r >= 0 and ki >= 4 * j:
                        nc.vector.tensor_mul(
                            exp_sb[:, off : off + 128],
                            exp_sb[:, off : off + 128],
                            md_sb[:],
                        )
                    elif mode == "general":
                        m_sb = mt_pool.tile([128, 512], F16, tag="mt")
                        nc.sync.dma_start(
                            out=m_sb[:],
                            in_=maskt[
                                ki * 128 : (ki + 1) * 128, j * 512 : (j + 1) * 512
                            ],
                        )
                        nc.vector.tensor_mul(exp_sb[:], exp_sb[:], m_sb[:])
                    # bf16 two-accumulator chain for the softmax denominator
                    a = accs[j]
                    w = a[2] % 2
                    if a[2] < 2:
                        t_acc = acc_pool.tile(
                            [128, 512], F16, tag=f"acc{w}", name=f"acc{pi}_{j}_{w}"
                        )
                        if off > 0:
                            nc.vector.memset(t_acc[:, :off], 0.0)
                        nc.vector.tensor_copy(t_acc[:, off:], exp_sb[:, off:])
                        a[w] = t_acc
                    else:
                        nc.vector.tensor_add(
                            a[w][:, off:], a[w][:, off:], exp_sb[:, off:]
                        )
                    a[2] += 1
                    if last and a[2] > 1:
                        # fold the two denominator accumulators eagerly so the
                        # ones-matmul in the (delayed) finish never waits on DVE
                        nc.vector.tensor_add(a[0][:], a[0][:], a[1][:])
                    pend.append((j, ki, off, exp_sb, first, last))
                    if len(pend) > AV_DELAY:
                        finish(pend.pop(0))
                    yield
                for stp in pend:
                    finish(stp)
                yield

        # ═══════ o_proj tile (b=0 staged from aots; b=1 direct SBUF) ═══════
        oproj_count = [0]

        def stage_block(jb):
            st = {}
            for hl in range(HPC):
                t_st = st_pool.tile(
                    [128, 512], F16, tag=f"st{hl}", name=f"st0{jb}{hl}"
                )
                nc.sync.dma_start(
                    out=t_st[:],
                    in_=aots[
                        hl * 128 : (hl + 1) * 128,
                        jb * 512 : (jb + 1) * 512,
                    ],
                )
                st[hl] = t_st
            return st

        def emit_oproj(st, b, mt, n):
            msl = slice(b * S + mt * 128, b * S + (mt + 1) * 128)
            ps = yp_pool.tile([128, 512], F32, tag="y")
            for hl in range(HPC):
                nc.tensor.matmul(
                    ps[:],
                    st[hl][:, (mt % 4) * 128 : (mt % 4 + 1) * 128],
                    wo_sb[:, hl * H + n * 512 : hl * H + (n + 1) * 512],
                    start=(hl == 0),
                    stop=(hl == HPC - 1),
                )
            y_sb = yo_pool.tile([128, 512], F16, tag="ysb")
            if oproj_count[0] % 2 == 0:
                nc.scalar.copy(y_sb[:], ps[:])
            else:
                nc.vector.tensor_copy(y_sb[:], ps[:])
            oproj_count[0] += 1
            nc.sync.dma_start(out=y[msl, n * 512 : (n + 1) * 512], in_=y_sb[:])

        # ═══════ Phase A (+R2): projections + RoPE, spill to DRAM ═══════
        with ExitStack() as actx:
            w_pool = actx.enter_context(tc.tile_pool(name="wqk", bufs=1))
            x_pool = actx.enter_context(tc.tile_pool(name="xblk", bufs=2))
            ev_pool = actx.enter_context(tc.tile_pool(name="evac", bufs=2))
            rp_pool = actx.enter_context(tc.tile_pool(name="rope", bufs=1))
            vh_pool = actx.enter_context(tc.tile_pool(name="vhl", bufs=1))
            ps_pool = actx.enter_context(
                tc.tile_pool(name="psA", bufs=3, space="PSUM")
            )

            wq_sb = w_pool.tile([128, KT * OC], F16, tag="wq")
            wk_sb = w_pool.tile([128, KT * OC], F16, tag="wk")
            wv_sb = w_pool.tile([128, KT * OC], F16, tag="wv")

            def load_x_quarter(tb, qi, nchunks=None):
                xh = x_pool.tile(
                    [128, KTQ * TB],
                    F16,
                    tag=f"xq{qi}",
                    name=f"x{tb}{qi}",
                    bufs=2 if qi < 3 else 1,
                )
                base = (tb * KT + qi * KTQ) * TB
                if nchunks is None:
                    nchunks = 2 if qi == 3 else 1  # xq3 is single-buffered
                csz = KTQ * TB // nchunks
                for c in range(nchunks):
                    # x loads ride the Activation DGE queue to stay clear of
                    # the spill traffic on the sync queue
                    nc.scalar.dma_start(
                        out=xh[:, c * csz : (c + 1) * csz],
                        in_=xtb[:, base + c * csz : base + (c + 1) * csz],
                    )
                return xh

            # start-ramp: wq + x(tb0) in fine chunks, consumption-ordered.
            # The first matmul needs only wq k-chunk 0 and x chunk 0, so the
            # leading loads are small; later loads coarsen.
            def wload(w_sb, wsrc, klo, khi):
                nc.sync.dma_start(
                    out=w_sb[:, klo * OC : khi * OC], in_=wsrc[:, klo * OC : khi * OC]
                )

            # x rides the Activation DGE stream, weights ride sync — two
            # independent descriptor pipes, both in consumption order with
            # small leading chunks so the first matmul starts early.
            x0q = [load_x_quarter(0, 0, nchunks=4)]
            wload(wq_sb, wq, 0, 1)
            wload(wq_sb, wq, 1, 4)
            x0q.append(load_x_quarter(0, 1, nchunks=2))
            wload(wq_sb, wq, 4, 8)
            wload(wq_sb, wq, 8, 16)
            x0q.append(load_x_quarter(0, 2, nchunks=2))
            cos_sb = const_pool.tile([HD, S], F16)
            nc.sync.dma_start(out=cos_sb[:], in_=cost[:])
            x0q.append(load_x_quarter(0, 3, nchunks=2))
            sin_sb = const_pool.tile([HD, S], F16)
            nc.sync.dma_start(out=sin_sb[:], in_=sinp[:])
            wload(wq_sb, wq, 16, 24)
            wload(wq_sb, wq, 24, 32)
            for c in range(4):
                wload(wk_sb, wk, c * 8, (c + 1) * 8)

            ones_sb = const_pool.tile([128, 128], F16)
            nc.sync.dma_start(out=ones_sb[:], in_=ones_t[:])
            if mode == "causal":
                md_sb = const_pool.tile([128, 128], F16)
                nc.sync.dma_start(out=md_sb[:], in_=mdiag[:])

            for c in range(4):
                # wv on the Activation stream behind x(tb0); needed ~27us in
                nc.scalar.dma_start(
                    out=wv_sb[:, c * 8 * OC : (c + 1) * 8 * OC],
                    in_=wv[:, c * 8 * OC : (c + 1) * 8 * OC],
                )

            def emit_tb(tb, xq=None):
                """Projections for t-block tb; yields after each of 13 groups."""
                if xq is None:
                    xq = [load_x_quarter(tb, qi) for qi in range(4)]
                xh = xq
                tsl = slice((tb % 4) * TB, (tb % 4 + 1) * TB)  # cos/sin cols
                gsl = slice(tb * TB, (tb + 1) * TB)            # global t cols
                def _qk_groups():
                    for which, w_sb, spill in (("q", wq_sb, qts), ("k", wk_sb, kts)):
                        for ot in range(HPC):
                            ps = ps_pool.tile([128, TB], F32, tag="proj")
                            for k in range(KT):
                                nc.tensor.matmul(
                                    ps[:],
                                    w_sb[:, k * OC + ot * 128 : k * OC + (ot + 1) * 128],
                                    xh[k // KTQ][:, (k % KTQ) * TB : (k % KTQ + 1) * TB],
                                    start=(k == 0),
                                    stop=(k == KT - 1),
                                )
                            raw = ev_pool.tile([128, TB], F16, tag="raw")
                            nc.scalar.copy(raw[:], ps[:])
                            # RoPE on DVE: t1 = raw*cos; t2 = rot_half(raw)*sin
                            # via partition-swapped reads of a sign-folded sin.
                            # The swapped reads come from PSUM (ps) — the DVE
                            # same-base-partition rule only binds SB+SB pairs.
                            # t2 first so ps's PSUM bank is released early.
                            t2 = rp_pool.tile([128, TB], F16, tag="t2")
                            nc.vector.tensor_mul(
                                t2[0:64, :], ps[64:128, :], sin_sb[0:64, tsl]
                            )
                            nc.vector.tensor_mul(
                                t2[64:128, :], ps[0:64, :], sin_sb[64:128, tsl]
                            )
                            t1 = rp_pool.tile([128, TB], F16, tag="t1")
                            nc.vector.tensor_mul(t1[:], raw[:], cos_sb[:, tsl])
                            nc.vector.tensor_add(raw[:], t1[:], t2[:])
                            nc.sync.dma_start(
                                out=spill[ot * 128 : (ot + 1) * 128, gsl], in_=raw[:]
                            )
                            yield
                def _v_groups():
                    vhl = [
                        vh_pool.tile([128, TB], F16, tag=f"vhl{hl}", name=f"vhl{hl}")
                        for hl in range(HPC)
                    ]
                    for mt in range(TB // 128):
                        ps = ps_pool.tile([128, OC], F32, tag="proj", name="psv")
                        for k in range(KT):
                            nc.tensor.matmul(
                                ps[:],
                                xh[k // KTQ][
                                    :, (k % KTQ) * TB + mt * 128 : (k % KTQ) * TB + (mt + 1) * 128
                                ],
                                wv_sb[:, k * OC : (k + 1) * OC],
                                start=(k == 0),
                                stop=(k == KT - 1),
                            )
                        for hl in range(HPC):
                            nc.scalar.copy(
                                vhl[hl][:, mt * 128 : (mt + 1) * 128],
                                ps[:, hl * 128 : (hl + 1) * 128],
                            )
                        yield
                    for hl in range(HPC):
                        nc.sync.dma_start(
                            out=vts[hl * 128 : (hl + 1) * 128, gsl], in_=vhl[hl][:]
                        )
                segs = (_v_groups, _qk_groups) if tb == 7 else (_qk_groups, _v_groups)
                for _seg in segs:
                    yield from _seg()

            PAIR_CHUNKS = sum(len(_kept(j, mode)) for j in range(NQ)) + 1
            B0_CHUNKS = HPC * PAIR_CHUNKS
            ag = attn_chain([(0, hl) for hl in range(HPC)])
            chunks = 0

            def pump(n_target):
                nonlocal chunks
                while chunks < n_target:
                    try:
                        next(ag)
                    except StopIteration:
                        return False
                    chunks += 1
                return True

            # A1: b=0 projections, dense; prime pair loads at the tail
            for tb in range(4):
                g = emit_tb(tb, x0q if tb == 0 else None)
                for _ in g:
                    pass
            next(ag)  # prime: emits first two pair loads only

            # R2: b=1 projections interleaved with b=0 attention
            groups = 0
            for tb in range(4, 8):
                for _ in emit_tb(tb):
                    groups += 1
                    pump(min((B0_CHUNKS * groups) // (4 * 12) + 1, B0_CHUNKS))
            # all b=1 spills are emitted now: safe to prime the b=1 chain;
            # its pair-0/1 loads overlap the b=0 attention drain below
            bg = attn_chain([(1, hl) for hl in range(HPC)])
            next(bg)

        # ═══════ R3: o_proj interleaved with remaining attention ═══════
        wo_pool = octx.enter_context(tc.tile_pool(name="wo", bufs=1))
        yo_pool = octx.enter_context(tc.tile_pool(name="yout", bufs=3))
        st_pool = octx.enter_context(tc.tile_pool(name="stage", bufs=3))
        spb_pool = octx.enter_context(tc.tile_pool(name="spb", bufs=1))
        yp_pool = octx.enter_context(tc.tile_pool(name="psY", bufs=3, space="PSUM"))

        wo_sb = wo_pool.tile([128, HPC * H], F16)
        nc.sync.dma_start(out=wo_sb[:], in_=wo[:])

        tiles_q = []

        def admit():
            while blk_unlocked:
                b, jb = blk_unlocked.pop(0)
                if b == 0:
                    st = stage_block(jb)
                else:
                    st = {hl: spb[(jb, hl)] for hl in range(HPC)}
                tiles_q.extend(
                    (st, b, 4 * jb + r, n)
                    for r in range(4)
                    for n in range(H // 512)
                )

        bchunks = 0
        emitted = 0
        # drain rest of b=0 attention, o_proj of unlocked b=0 blocks mixed in
        while chunks < B0_CHUNKS:
            try:
                next(ag)
            except StopIteration:
                break
            chunks += 1
            bchunks += 1
            admit()
            if bchunks > 12:
                while emitted < bchunks - 12 and len(tiles_q) > 4:
                    emit_oproj(*tiles_q.pop(0))
                    emitted += 1
        bg_done = False
        while not bg_done:
            try:
                next(bg)
                bchunks += 1
            except StopIteration:
                bg_done = True
                break
            admit()
            if bchunks > 12:
                while emitted < bchunks - 12 and len(tiles_q) > 4:
                    emit_oproj(*tiles_q.pop(0))
                    emitted += 1
        admit()
        for st_b_mt_n in tiles_q:
            emit_oproj(*st_b_mt_n)

    return nc


_CACHE: dict = {}


def _get_nc(mode: str) -> bass.Bass:
    if mode not in _CACHE:
        _CACHE[mode] = _build(mode)
    return _CACHE[mode]


def _rope_tables():
    inv_freq = 1.0 / (THETA ** (np.arange(0, HD, 2, dtype=np.float32) / HD))
    t = np.arange(S, dtype=np.float32)
    freqs = np.einsum("i,j->ij", t, inv_freq)
    emb = np.concatenate((freqs, freqs), axis=-1)  # [S, HD]
    return np.cos(emb), np.sin(emb)


def kernel(hidden_states, attention_mask, Wq, Wk, Wv, Wo):
    hs = np.asarray(hidden_states, dtype=np.float32)
    mask = np.asarray(attention_mask, dtype=np.float32)[0, 0]
    Wq = np.asarray(Wq, dtype=np.float32)
    Wk = np.asarray(Wk, dtype=np.float32)
    Wv = np.asarray(Wv, dtype=np.float32)
    Wo = np.asarray(Wo, dtype=np.float32)

    causal = np.triu(np.full((S, S), -1e9, dtype=np.float32), k=1)
    if np.array_equal(mask, causal):
        mode = "causal"
    elif not mask.any():
        mode = "zeros"
    else:
        mode = "general"

    # ── host-side prep (blocked layouts) ──
    xt = hs.reshape(T, H).T                       # [H, T] fp32
    # xtb[p, (tb k t)] = xt[k*128+p, tb*512+t]
    xtb = np.ascontiguousarray(
        xt.reshape(KT, 128, NTB, TB).transpose(1, 2, 0, 3).reshape(128, NTB * KT * TB)
    ).astype(BF)
    cos, sin = _rope_tables()                     # [S, HD] fp32
    cost = np.ascontiguousarray(cos.T).astype(BF)   # [HD, S]
    sinf = sin.T.copy()                             # [HD, S], sign-folded
    sinf[: HD // 2] *= -1.0
    sinp = np.ascontiguousarray(sinf).astype(BF)
    ones_t = np.ones((128, 128), dtype=BF)

    common = {"cost": cost, "sinp": sinp, "ones_t": ones_t, "xtb": xtb}
    if mode == "causal":
        p_idx = np.arange(128)[:, None]
        c_idx = np.arange(128)[None, :]
        md = np.where(p_idx > c_idx, np.float32(0), np.float32(1))
        common["mdiag"] = np.ascontiguousarray(md).astype(BF)
    elif mode == "general":
        common["maskt"] = np.ascontiguousarray(
            np.exp(np.clip(mask.T.astype(np.float64), -80, 11))
        ).astype(BF)

    def wblock(Wslice):  # [OC rows of W, H] -> [128, KT*OC] (k, oc)
        wt = Wslice.T  # [H, OC]
        return np.ascontiguousarray(
            wt.reshape(KT, 128, OC).transpose(1, 0, 2).reshape(128, KT * OC)
        ).astype(BF)

    in_maps = []
    for c in range(NCORES):
        osl = slice(OC * c, OC * (c + 1))
        wot = Wo[:, osl].T  # [OC, H]
        wob = np.ascontiguousarray(
            wot.reshape(HPC, 128, H).transpose(1, 0, 2).reshape(128, HPC * H)
        ).astype(BF)
        in_maps.append(
            dict(
                common,
                wq=wblock(Wq[osl, :]),
                wk=wblock(Wk[osl, :]),
                wv=wblock(Wv[osl, :]),
                wo=wob,
            )
        )

    global _last_in_maps
    _last_in_maps = in_maps
    nc = _get_nc(mode)
    res = run_bass_kernel_spmd(nc, in_maps, list(range(NCORES)))
    out = np.zeros((T, H), dtype=np.float32)
    for c in range(NCORES):
        out += res.results[c]["y"].astype(np.float32)
    return out.reshape(B, S, H)
